# revision 1
# baseline (speedup 1.0000x reference)
"""BiLSTM decoder model — Trainium2 Bass kernel, 8 NeuronCores, batch-parallel.

Math (from the reference):
  emb = tanh(embed[seq])                       # [B, S, E]
  f_hs = LSTM_fwd(emb over t)                  # only f_hs[-1] used
  b_hs[0] = one LSTM cell step of the backward LSTM on emb[:, 0]
            (the reference's backward scan output is only read at index 0,
             which depends solely on the first scanned element emb[0])
  seq_repr = [f_h_last, b_h0]                  # [B, 2H]
  out = log_softmax(seq_repr @ Wc.T + bc)      # [B, 5]

Sharding: data-parallel over batch. 8 cores x 32 rows each; LSTM weights
replicated. Per core:
  Phase A: indirect-DMA gather of embedding rows (token-major), tanh (ACT),
           PE transpose to x^T layout, input-projection matmuls (fp16) for all
           timesteps -> Wx staged in DRAM; plus the single backward cell.
  Phase B: 256-step recurrence. Whh (fp16, gate-permuted [i|g|f|o]) stationary
           tiles; h^T is the moving operand (N=32). Gates/state on ACT+DVE
           overlap the matmul stream. c kept fp32; h fp16.
  Logits:  fp32 matmul [32,2048]@[2048,5] + bias + log_softmax on device.

Everything here is hardcoded for B=256, S=256, E=512, H=1024, V_OUT=5.
"""

import os

import numpy as np

P = 128
B = 256
S_FULL = 256
E = 512
H = 1024
G4 = 4 * H            # 4096 gate rows
V_OUT = 5
NCORES = 8
BC = B // NCORES      # 32 batch rows per core
KE = E // P           # 4 k-tiles over E
KH = H // P           # 8 k-tiles over H
M4 = G4 // P          # 32 m-tiles over gates
CH = 16               # scan steps per Wx chunk

_PROGRAMS = {}        # S -> (nc, meta)


def _gate_perm():
    # reorder 4H rows from [i f g o] to [i g f o] so that psum bank0 = {i,g}
    # and bank1 = {f,o}; within each block natural H order is kept.
    return np.concatenate([
        np.arange(0, H),
        np.arange(2 * H, 3 * H),
        np.arange(H, 2 * H),
        np.arange(3 * H, 4 * H),
    ])


def _tile_kxm(w_t, k_tiles):
    # w_t: [K, M4*P] -> sbuf layout [128, k_tiles * M] with tile k at cols
    # [k*M : (k+1)*M]; element [p, k*M + j] = w_t[k*128 + p, j]
    K, M = w_t.shape
    assert K == k_tiles * P
    return np.ascontiguousarray(
        w_t.reshape(k_tiles, P, M).transpose(1, 0, 2).reshape(P, k_tiles * M)
    )


def _build_program(S):
    from contextlib import ExitStack

    from concourse import bacc, bass, mybir, tile

    f16 = mybir.dt.float16
    f32 = mybir.dt.float32
    i32 = mybir.dt.int32
    AF = mybir.ActivationFunctionType
    ALU = mybir.AluOpType
    AX = mybir.AxisListType

    TOK = S * BC                 # tokens per core
    NJ = TOK // P                # gather blocks of 128 tokens
    NQ = 4 if NJ % 4 == 0 else 1   # token quarters for pipelining
    TOKQ = TOK // NQ
    NJQ = NJ // NQ
    CHS = min(CH, S)
    n_chunks = (S + CHS - 1) // CHS

    nc = bacc.Bacc("TRN2", debug=False, enable_asserts=False,
                   num_devices=NCORES)

    embed_h = nc.dram_tensor("embed", [50257, E], f32, kind="ExternalInput")
    seqidx_h = nc.dram_tensor("seqidx", [P, NJ], i32, kind="ExternalInput")
    wihT_h = nc.dram_tensor("wihT", [P, KE * G4], f16, kind="ExternalInput")
    wbT_h = nc.dram_tensor("wbT", [P, KE * G4], f16, kind="ExternalInput")
    whhT_h = nc.dram_tensor("whhT", [P, KH * G4], f16, kind="ExternalInput")
    bfT_h = nc.dram_tensor("bfT", [P, M4], f32, kind="ExternalInput")
    bbT_h = nc.dram_tensor("bbT", [P, M4], f32, kind="ExternalInput")
    wcT_h = nc.dram_tensor("wcT", [P, 16 * V_OUT], f32, kind="ExternalInput")
    bc_h = nc.dram_tensor("bcv", [1, V_OUT], f32, kind="ExternalInput")
    ident_h = nc.dram_tensor("ident", [P, P], f16, kind="ExternalInput")
    out_h = nc.dram_tensor("out", [BC, V_OUT], f32, kind="ExternalOutput")
    debug = bool(int(os.environ.get("TRN_KERNEL_DEBUG", "0")))
    if debug:
        hf_out = nc.dram_tensor("hf_out", [P, KH, BC], f32,
                                kind="ExternalOutput")
        hb_out = nc.dram_tensor("hb_out", [P, KH, BC], f32,
                                kind="ExternalOutput")
        xt_dbg = nc.dram_tensor("xt_dbg", [P, KE, S * BC], f16,
                                kind="ExternalOutput")
        wx_dbg = nc.dram_tensor("wx_dbg", [M4, P, S * BC], f16,
                                kind="ExternalOutput")
    # Wx scratch in DRAM: per m-tile, all tokens contiguous.
    wx_h = nc.dram_tensor("wxscratch", [M4, P, TOK], f16, kind="Internal")

    embed = embed_h.ap()
    wx_d = wx_h.ap()

    with tile.TileContext(nc, trace_sim=False) as tc, ExitStack() as ctx:
        # ---- pools that live for the whole kernel ----
        cpool = ctx.enter_context(tc.tile_pool(name="const", bufs=1))
        keep = ctx.enter_context(tc.tile_pool(name="keep", bufs=1))

        seqidx = cpool.tile([P, NJ], i32)
        nc.gpsimd.dma_start(seqidx[:], seqidx_h.ap()[:])
        ident = cpool.tile([P, P], f16)
        nc.sync.dma_start(ident[:], ident_h.ap()[:])
        bfT = cpool.tile([P, M4], f32)
        nc.sync.dma_start(bfT[:], bfT_h.ap()[:])
        bbT = cpool.tile([P, M4], f32)
        nc.sync.dma_start(bbT[:], bbT_h.ap()[:])
        wcT = cpool.tile([P, 16 * V_OUT], f32)
        nc.sync.dma_start(wcT[:], wcT_h.ap()[:])
        bcv = cpool.tile([1, V_OUT], f32)
        nc.sync.dma_start(bcv[:], bc_h.ap()[:])
        ones32 = cpool.tile([1, BC], f32)
        nc.vector.memset(ones32[:], 1.0)

        h_b = keep.tile([P, KH, BC], f32)      # backward-cell hidden
        h_f = keep.tile([P, KH, BC], f32)      # final forward hidden

        # ================= Phase A =================
        _KEEP_OPEN = bool(int(os.environ.get("TRN_KEEP_OPEN", "0")))
        actx = ExitStack()
        if _KEEP_OPEN:
            ctx.enter_context(actx)
        if True:
            apool_w = actx.enter_context(tc.tile_pool(name="aw", bufs=1))
            wihT = apool_w.tile([P, KE * G4], f16)
            nc.sync.dma_start(wihT[:], wihT_h.ap()[:])
            wbT = apool_w.tile([P, KE * G4], f16)
            nc.sync.dma_start(wbT[:], wbT_h.ap()[:])

            # x^T per token-quarter: [128, KE, TOKQ] fp16
            xt_pool = actx.enter_context(tc.tile_pool(name="xt", bufs=1))
            xT = [xt_pool.tile([P, KE, TOKQ], f16, name=f"xT{q}")
                  for q in range(NQ)]

            gpool = actx.enter_context(tc.tile_pool(name="gath", bufs=3))
            gtpool = actx.enter_context(tc.tile_pool(name="gtan", bufs=3))
            tpsum = actx.enter_context(
                tc.tile_pool(name="tpsum", bufs=1 if _KEEP_OPEN else 2, space="PSUM"))
            xpsum = actx.enter_context(
                tc.tile_pool(name="xpsum", bufs=1 if _KEEP_OPEN else 3, space="PSUM"))
            bpsum = actx.enter_context(
                tc.tile_pool(name="bpsum", bufs=1, space="PSUM"))
            stg = actx.enter_context(tc.tile_pool(name="stg", bufs=4))

            # gather + tanh + transpose; GN tokens per partition per call
            GN = 1  # indirect DMA fetches one row per partition per call
            for j in range(NJ // GN):
                g = gpool.tile([P, GN, E], f32)
                nc.gpsimd.indirect_dma_start(
                    out=g[:].rearrange("p n e -> p (n e)") if GN > 1 else
                    g[:, 0, :],
                    out_offset=None, in_=embed[:],
                    in_offset=bass.IndirectOffsetOnAxis(
                        ap=seqidx[:, j * GN:(j + 1) * GN], axis=0),
                )
                gt = gtpool.tile([P, GN, E], f16)
                nc.scalar.activation(gt[:], g[:], AF.Tanh)
                for n in range(GN):
                    jj = j * GN + n
                    q, jq = divmod(jj, NJQ)
                    ps = tpsum.tile([P, E], f16)
                    for c in range(KE):
                        nc.tensor.transpose(ps[:, c * P:(c + 1) * P],
                                            gt[:, n, c * P:(c + 1) * P],
                                            ident[:])
                    for c in range(KE):
                        nc.vector.tensor_copy(
                            xT[q][:, c, jq * P:(jq + 1) * P],
                            ps[:, c * P:(c + 1) * P])

            # forward input projection for all tokens -> DRAM
            for m in range(M4):
                for q in range(NQ):
                    for n0 in range(0, TOKQ, 512):
                        w = min(512, TOKQ - n0)
                        ps = xpsum.tile([P, 512], f32)
                        for k in range(KE):
                            nc.tensor.matmul(
                                ps[:, :w],
                                wihT[:, k * G4 + m * P: k * G4 + (m + 1) * P],
                                xT[q][:, k, n0:n0 + w],
                                start=(k == 0), stop=(k == KE - 1))
                        st = stg.tile([P, 512], f16)
                        if m % 2 == 0:
                            nc.scalar.activation(st[:, :w], ps[:, :w],
                                                 AF.Identity,
                                                 bias=bfT[:, m:m + 1])
                        else:
                            nc.vector.tensor_scalar(
                                st[:, :w], ps[:, :w], bfT[:, m:m + 1], None,
                                op0=ALU.add)
                        nc.sync.dma_start(
                            wx_d[m, :, q * TOKQ + n0: q * TOKQ + n0 + w],
                            st[:, :w])

            # backward cell on t=0 tokens (cols 0:32 of quarter 0)
            psb = bpsum.tile([P, M4, BC], f32)
            for m in range(M4):
                for k in range(KE):
                    nc.tensor.matmul(
                        psb[:, m, :],
                        wbT[:, k * G4 + m * P: k * G4 + (m + 1) * P],
                        xT[0][:, k, 0:BC],
                        start=(k == 0), stop=(k == KE - 1))
            gb = stg.tile([P, M4, BC], f16)
            for m in range(M4):
                func = AF.Tanh if 8 <= m < 16 else AF.Sigmoid
                nc.scalar.activation(gb[:, m, :], psb[:, m, :], func,
                                     bias=bbT[:, m:m + 1])
            if debug:
                for q in range(NQ):
                    nc.sync.dma_start(
                        xt_dbg.ap()[:, :, q * TOKQ:(q + 1) * TOKQ], xT[q][:])
                for m in range(M4):
                    nc.sync.dma_start(wx_dbg.ap()[m], wx_d[m])
            cb = stg.tile([P, KH, BC], f32)
            nc.vector.tensor_mul(cb[:], gb[:, 0:KH, :], gb[:, KH:16, :])
            tcb = stg.tile([P, KH, BC], f32)
            nc.scalar.activation(tcb[:], cb[:], AF.Tanh)
            nc.vector.tensor_mul(h_b[:], gb[:, 24:32, :], tcb[:])

        if not _KEEP_OPEN:
            actx.close()
        # ================= Phase B: the scan =================
        tc.strict_bb_all_engine_barrier()
        bpool_w = ctx.enter_context(tc.tile_pool(name="bw", bufs=1))
        whhT = bpool_w.tile([P, KH * G4], f16)
        nc.sync.dma_start(whhT[:], whhT_h.ap()[:])

        state = ctx.enter_context(tc.tile_pool(name="state", bufs=1))
        hping = [state.tile([P, KH, BC], f16, name=f"h{i}") for i in range(2)]
        cping = [state.tile([P, KH, BC], f32, name=f"c{i}") for i in range(2)]
        nc.vector.memset(hping[0][:], 0.0)
        nc.vector.memset(cping[0][:], 0.0)

        wxpool = ctx.enter_context(tc.tile_pool(name="wxc", bufs=2))
        gsl = ctx.enter_context(tc.tile_pool(name="gsl", bufs=2))
        tmp = ctx.enter_context(tc.tile_pool(name="tmp", bufs=2))

        sctx = ExitStack()
        # one PSUM bank per gate region [i|g|f|o] so the c-path (needs i,g,f)
        # runs while the o-region matmuls are still streaming
        spsum = sctx.enter_context(
            tc.tile_pool(name="spsum", bufs=2, space="PSUM"))

        chunk = None
        for t in range(S):
            hp = hping[t % 2]
            hn = hping[(t + 1) % 2]
            cp = cping[t % 2]
            cn = cping[(t + 1) % 2]
            if t % CHS == 0:
                cc = t // CHS
                w = min(CHS, S - cc * CHS) * BC
                chunk = wxpool.tile([P, M4, CHS * BC], f16)
                for m in range(M4):
                    nc.sync.dma_start(
                        chunk[:, m, :w],
                        wx_d[m, :, cc * CHS * BC: cc * CHS * BC + w])
            toff = (t % CHS) * BC

            # [128, 4 regions (bank-padded to 16 m-slots), 32]
            ps = spsum.tile([P, 4, 16, BC], f32)
            a = gsl.tile([P, M4, BC], f16, tag="a")
            g = gsl.tile([P, M4, BC], f16, tag="g")
            t1 = tmp.tile([P, KH, BC], f32, tag="t1")
            cm = tmp.tile([P, KH, BC], f32, tag="cm")
            tch = tmp.tile([P, KH, BC], f16, tag="tch")
            for r, func in ((0, AF.Sigmoid), (1, AF.Tanh),
                            (2, AF.Sigmoid), (3, AF.Sigmoid)):
                for mm in range(8):
                    for k in range(KH):
                        nc.tensor.matmul(
                            ps[:, r, mm, :],
                            whhT[:, k * G4 + (8 * r + mm) * P:
                                 k * G4 + (8 * r + mm + 1) * P],
                            hp[:, k, :],
                            start=(k == 0), stop=(k == KH - 1))
                sl = slice(8 * r, 8 * (r + 1))
                nc.vector.tensor_add(
                    a[:, sl, :], ps[:, r, 0:8, :],
                    chunk[:, sl, toff:toff + BC])
                nc.scalar.activation(g[:, sl, :], a[:, sl, :], func)
                if r == 1:
                    nc.vector.tensor_mul(t1[:], g[:, 0:8, :], g[:, 8:16, :])
                elif r == 2:
                    nc.vector.tensor_mul(cm[:], g[:, 16:24, :], cp[:])
                    nc.vector.tensor_add(cn[:], cm[:], t1[:])
                    nc.scalar.activation(tch[:], cn[:], AF.Tanh)

            nc.vector.tensor_mul(hn[:], g[:, 24:32, :], tch[:])
            if t == S - 1:
                nc.vector.tensor_mul(h_f[:], g[:, 24:32, :], tch[:])

        sctx.close()
        # ================= logits + log_softmax =================
        lpsum = ctx.enter_context(
            tc.tile_pool(name="lpsum", bufs=1, space="PSUM"))
        lp = lpsum.tile([BC, V_OUT], f32)
        for k in range(KH):
            nc.tensor.matmul(lp[:], h_f[:, k, :], wcT[:, k * V_OUT:(k + 1) * V_OUT],
                             start=(k == 0), stop=False)
        for k in range(KH):
            nc.tensor.matmul(lp[:], h_b[:, k, :],
                             wcT[:, (KH + k) * V_OUT:(KH + k + 1) * V_OUT],
                             start=False, stop=False)
        nc.tensor.matmul(lp[:], ones32[:], bcv[:], start=False, stop=True)

        fin = ctx.enter_context(tc.tile_pool(name="fin", bufs=1))
        if debug:
            nc.sync.dma_start(hf_out.ap()[:], h_f[:])
            nc.sync.dma_start(hb_out.ap()[:], h_b[:])
        mx = fin.tile([BC, 1], f32)
        nc.vector.reduce_max(mx[:], lp[:], axis=AX.X)
        e = fin.tile([BC, V_OUT], f32)
        nc.vector.tensor_scalar(e[:], lp[:], mx[:], None, op0=ALU.subtract)
        ex = fin.tile([BC, V_OUT], f32)
        se = fin.tile([BC, 1], f32)
        nc.scalar.activation(ex[:], e[:], AF.Exp, accum_out=se[:])
        ls = fin.tile([BC, 1], f32)
        nc.scalar.activation(ls[:], se[:], AF.Ln)
        res = fin.tile([BC, V_OUT], f32)
        nc.vector.tensor_scalar(res[:], e[:], ls[:], None, op0=ALU.subtract)
        nc.sync.dma_start(out_h.ap()[:], res[:])

    nc.compile()
    return nc


def _get_program(S):
    if S not in _PROGRAMS:
        _PROGRAMS[S] = _build_program(S)
    return _PROGRAMS[S]


def _prep_inputs(seq, embed, Wf_ih, Wf_hh, bf_ih, bf_hh, Wb_ih, Wb_hh,
                 bb_ih, bb_hh, Wc, bc):
    seq = np.asarray(seq)
    Bsz, S = seq.shape
    assert Bsz == B
    f16 = np.float16
    perm = _gate_perm()

    embed = np.ascontiguousarray(np.asarray(embed, np.float32))
    wihT = _tile_kxm(np.asarray(Wf_ih, np.float32)[perm].T.astype(f16), KE)
    wbT = _tile_kxm(np.asarray(Wb_ih, np.float32)[perm].T.astype(f16), KE)
    whhT = _tile_kxm(np.asarray(Wf_hh, np.float32)[perm].T.astype(f16), KH)
    bf = (np.asarray(bf_ih, np.float32) + np.asarray(bf_hh, np.float32))[perm]
    bb = (np.asarray(bb_ih, np.float32) + np.asarray(bb_hh, np.float32))[perm]
    bfT = np.ascontiguousarray(bf.reshape(M4, P).T)
    bbT = np.ascontiguousarray(bb.reshape(M4, P).T)
    wcT = np.ascontiguousarray(
        np.asarray(Wc, np.float32).T.reshape(16, P, V_OUT)
        .transpose(1, 0, 2).reshape(P, 16 * V_OUT))
    bcv = np.asarray(bc, np.float32).reshape(1, V_OUT)
    ident = np.eye(P, dtype=f16)

    shared = dict(embed=embed, wihT=wihT, wbT=wbT, whhT=whhT, bfT=bfT,
                  bbT=bbT, wcT=wcT, bcv=bcv, ident=ident)
    in_maps = []
    NJ = (S * BC) // P
    for c in range(NCORES):
        tok = seq[c * BC:(c + 1) * BC, :].T.reshape(-1)   # token-major
        seqidx = np.ascontiguousarray(
            tok.reshape(NJ, P).T.astype(np.int32))
        in_maps.append(dict(shared, seqidx=seqidx))
    return in_maps, S


def _run(in_maps, S, trace=False):
    from concourse.bass_utils import run_bass_kernel_spmd

    if trace:
        # the agent image's antenv lacks axon_hooks; provide a stub so
        # run_bass_kernel_spmd degrades gracefully instead of crashing
        import sys
        import types
        try:
            from antenv import axon_hooks  # noqa: F401
        except ImportError:
            mod = types.ModuleType("antenv.axon_hooks")
            mod.get_axon_ntff_profile_hook = lambda: None
            mod.set_axon_ntff_profile_hook = lambda h: None
            sys.modules["antenv.axon_hooks"] = mod

    nc = _get_program(S)
    return run_bass_kernel_spmd(nc, in_maps, core_ids=list(range(NCORES)),
                                trace=trace)


def kernel(**inputs) -> np.ndarray:
    in_maps, S = _prep_inputs(**inputs)
    trace = bool(int(os.environ.get("TRN_KERNEL_TRACE", "0")))
    r = _run(in_maps, S, trace=trace)
    kernel.last_results = r
    out = np.concatenate([res["out"] for res in r.results], axis=0)
    return out.astype(np.float32)

